# revision 18
# baseline (speedup 1.0000x reference)
"""Trainium2 Bass kernel for nn_Pooling_Layer (GNN message-passing pooling).

Math (per batch element b):
    x = in_pc_pad[b] @ weight_res.T               # (N+1, 64) -> (N+1, 128) projection
    w = |p_neighbors| * mask; w /= w.sum(-1)+1e-8 # (P, 32) pooling weights
    out[b, p] = sum_m w[p, m] * x[id[p, m]]       # gather + weighted pool

We reorder: pool first in C_IN=64 space (gather is half the bytes), then
project pooled (P, 64) @ weight_res.T.  Normalization (divide by the weight
sum) is folded into the PSUM->SBUF copy after the projection.

Sharding: points are sharded across the 8 cores (1250 points each); every
core handles ALL batches for its points.  The gather table holds row PAIRS,
batch-interleaved, in bf16: xi[k] = [row 2k: b0..b7 x 64ch | row 2k+1:
b0..b7 x 64ch] (2KB rows).  Pairs keep the SWDGE gather indices int16-safe
(idx = id >> 1 <= 20000); one descriptor serves all 8 batches at a
DMA-efficient 2KB.  bf16 halves HBM gather traffic vs f32; the tolerance
(2e-2) dwarfs bf16 rounding (~0.5%).

Pooling runs on the TensorEngine: per 128-point tile, 64 accumulating bf16
matmuls (32 windows x even/odd half) into one (128 pts, 8b*64ch) PSUM bank.
lhsT is a block-diagonal weight matrix with a FIXED sparsity structure:
window w (slots = partitions: slot 32q+m = neighbor m of point 4w+q) puts
weight at [32q+m, 4w+q].  Even-half weights are |pn|*mask*(1-parity), odd
|pn|*mask*parity, so the wrong half of each gathered pair contributes 0.
The nonzero positions are identical for every tile, so the bd buffers are
zeroed once and only the values are rewritten per tile (tiny strided
copies).

Then per tile: 4 PE transposes (128pts, 2 batches*64ch) -> (128ch, 128pts),
8 projection matmuls lhsT=pooled^T (64,128) rhs=weight_res^T (64,128), and
the per-point 1/denom scale on the PSUM->SBUF copy.  Output is bf16,
upcast and re-assembled on the host.
"""

import numpy as np
import ml_dtypes

import concourse.bass as bass
import concourse.mybir as mybir
import concourse.tile as tile
from concourse import bacc, library_config
from concourse.bass_utils import run_bass_kernel_spmd

F32 = mybir.dt.float32
BF16 = mybir.dt.bfloat16
I16 = mybir.dt.int16

MAXN = 32
CIN = 64
COUT = 128
B = 8
IN_ROWS = 40001          # in_pc_pad rows (incl. pad row)
NPAIRS = 20001           # row pairs (rows padded to 40002)
EW = B * CIN             # interleaved single-row width (elements) = 512
PEW = 2 * EW             # pair-row width = 1024 elements (2KB bf16)
PTS = 10000
NWIN = 32                # windows (4-point groups) per 128-point tile
CHW = 8                  # windows per gather call (1024 idx)
NCALL = (NWIN + CHW - 1) // CHW   # gather call slots per tile


class Params:
    def __init__(self, pts=PTS, n_cores=8):
        self.pts = pts
        self.n_cores = n_cores
        self.cpts = pts // n_cores            # points per core (1250)
        self.ntl = (self.cpts + 127) // 128   # 128-point tiles per core (10)
        self.cpts_pad = self.ntl * 128        # 1280


def build_nc(p: Params):
    nc = bacc.Bacc(
        "TRN2",
        target_bir_lowering=False,
        debug=False,
        num_devices=p.n_cores,
        num_swdge_queues=4,
    )
    NTL = p.ntl
    xi = nc.dram_tensor("xi", [NPAIRS, PEW], BF16, kind="ExternalInput")
    idxw = nc.dram_tensor("idxw", [128, NTL * NWIN * 8], I16, kind="ExternalInput")
    pnT = nc.dram_tensor("pnT", [128, p.cpts_pad], F32, kind="ExternalInput")
    maskT = nc.dram_tensor("maskT", [128, p.cpts_pad], F32, kind="ExternalInput")
    parT = nc.dram_tensor("parT", [128, p.cpts_pad], F32, kind="ExternalInput")
    pnN = nc.dram_tensor("pnN", [p.cpts_pad, MAXN], F32, kind="ExternalInput")
    maskN = nc.dram_tensor("maskN", [p.cpts_pad, MAXN], F32, kind="ExternalInput")
    wres = nc.dram_tensor("wres", [COUT, CIN], F32, kind="ExternalInput")
    ident = nc.dram_tensor("ident", [128, 128], F32, kind="ExternalInput")
    out = nc.dram_tensor("out", [B * p.cpts_pad, COUT], BF16, kind="ExternalOutput")

    with tile.TileContext(nc) as tc:
        with (
            tc.tile_pool(name="const", bufs=1) as constp,
            tc.tile_pool(name="prep", bufs=1) as prep,
            tc.tile_pool(name="gather", bufs=6) as gp,
            tc.tile_pool(name="work", bufs=2) as wk,
            tc.tile_pool(name="psP", bufs=2, space="PSUM") as psP,
            tc.tile_pool(name="psT", bufs=2, space="PSUM") as psT,
            tc.tile_pool(name="psO", bufs=2, space="PSUM") as psO,
        ):
            nc.gpsimd.load_library(library_config.mlp)

            # ---- constants ----
            identity = constp.tile([128, 128], F32)
            nc.sync.dma_start(out=identity[:], in_=ident[:])
            wres_sb = constp.tile([COUT, CIN], F32)
            nc.sync.dma_start(out=wres_sb[:], in_=wres[:])
            psw = psT.tile([CIN, COUT], F32, tag="psTt")
            nc.tensor.transpose(out=psw[:], in_=wres_sb[:], identity=identity[:])
            # [i, o] = wres[o, i], replicated into both 64-partition halves so
            # the projection matmul's rhs base partition matches lhsT's
            wresTb = constp.tile([128, COUT], BF16)
            nc.vector.tensor_copy(out=wresTb[0:CIN, :], in_=psw[:])
            nc.vector.tensor_copy(out=wresTb[CIN : 2 * CIN, :], in_=psw[:])

            # idx loaded per tile so the first gather starts immediately
            idx_sb = constp.tile([128, NTL * NWIN * 8], I16)
            for t in range(NTL):
                c0 = t * NWIN * 8
                nc.sync.dma_start(
                    out=idx_sb[:, c0 : c0 + NWIN * 8],
                    in_=idxw[:, c0 : c0 + NWIN * 8],
                )

            # ---- per-point reciprocal denominators: recip[p, t] ----
            prodN = prep.tile([128, NTL * MAXN], F32)
            nc.sync.dma_start(
                out=prodN[:].rearrange("p (t m) -> p t m", m=MAXN),
                in_=pnN[:].rearrange("(t p) m -> p t m", p=128),
            )
            maskN_sb = prep.tile([128, NTL * MAXN], F32)
            nc.sync.dma_start(
                out=maskN_sb[:].rearrange("p (t m) -> p t m", m=MAXN),
                in_=maskN[:].rearrange("(t p) m -> p t m", p=128),
            )
            nc.vector.tensor_tensor(
                out=prodN[:], in0=prodN[:], in1=maskN_sb[:], op=mybir.AluOpType.mult
            )
            denom = constp.tile([128, NTL], F32)
            nc.vector.tensor_reduce(
                out=denom[:],
                in_=prodN[:].rearrange("p (t m) -> p t m", m=MAXN),
                op=mybir.AluOpType.add,
                axis=mybir.AxisListType.X,
                apply_absolute_value=True,
            )
            nc.vector.tensor_scalar_add(denom[:], denom[:], 1e-8)
            recip = constp.tile([128, NTL], F32)
            nc.vector.reciprocal(out=recip[:], in_=denom[:])

            # ---- pooling weights in (32q+m, pt) layout ----
            # wsel0 = |pn|*mask*(1-par)   (even half)
            # wsel1 = |pn|*mask*par       (odd half)
            pnT_sb = prep.tile([128, p.cpts_pad], F32)
            maskT_sb = prep.tile([128, p.cpts_pad], F32)
            parT_sb = prep.tile([128, p.cpts_pad], F32)
            nc.sync.dma_start(out=pnT_sb[:], in_=pnT[:])
            nc.sync.dma_start(out=maskT_sb[:], in_=maskT[:])
            nc.sync.dma_start(out=parT_sb[:], in_=parT[:])
            wsel0 = prep.tile([128, p.cpts_pad], F32)
            wsel1 = prep.tile([128, p.cpts_pad], F32)
            nc.scalar.activation(
                out=wsel0[:], in_=pnT_sb[:], func=mybir.ActivationFunctionType.Abs
            )
            nc.vector.tensor_tensor(
                out=wsel0[:], in0=wsel0[:], in1=maskT_sb[:], op=mybir.AluOpType.mult
            )
            nc.vector.tensor_tensor(
                out=wsel1[:], in0=wsel0[:], in1=parT_sb[:], op=mybir.AluOpType.mult
            )
            nc.vector.tensor_tensor(
                out=wsel0[:], in0=wsel0[:], in1=wsel1[:], op=mybir.AluOpType.subtract
            )

            # ---- block-diag weight buffers: fixed sparsity, zeroed once ----
            BDW = NWIN * 132  # 4224: bd[s, 132w + q] == lhsT col 4w+q of window w
            bd_bufs = []      # [t%2][half] ping-pong pairs
            for i in range(2):
                pair = []
                for half in range(2):
                    bdt = constp.tile([128, BDW], BF16, tag=f"bd{i}h{half}")
                    nc.vector.memset(bdt[:], 0.0)
                    pair.append(bdt)
                bd_bufs.append(pair)

            # ---- main loop over 128-point tiles ----
            for t in range(NTL):
                # windows with at least one real (non-pad) point
                real_pts = min(128, p.cpts - t * 128)
                nwin_t = (real_pts + 3) // 4
                # scatter this tile's weights onto the fixed block-diag slots
                bde, bdo = bd_bufs[t % 2]
                for bd, src in ((bde, wsel0), (bdo, wsel1)):
                    bdv = bd[:].rearrange("p (w c) -> p w c", c=132)
                    sv = src[:, t * 128 : (t + 1) * 128].rearrange(
                        "p (w four) -> p w four", four=4
                    )
                    for q in range(4):
                        nc.vector.tensor_copy(
                            out=bdv[32 * q : 32 * q + 32, :, q],
                            in_=sv[32 * q : 32 * q + 32, :, q],
                        )

                # gather + pool in chunks of 8 windows (1024 idx per call)
                ps = psP.tile([128, EW], F32, tag="ps")
                for c in range((nwin_t + CHW - 1) // CHW):
                    nw_c = min(CHW, nwin_t - c * CHW)
                    g = gp.tile([128, CHW * PEW], BF16, tag="g")
                    call = t * NCALL + c
                    col0 = t * NWIN * 8 + c * CHW * 8
                    nc.gpsimd.dma_gather(
                        g[:, : nw_c * PEW].rearrange("p (v e) -> p v e", e=PEW),
                        xi[:],
                        idx_sb[:, col0 : col0 + nw_c * 8],
                        nw_c * 128,
                        nw_c * 128,
                        PEW,
                        queue_num=call % 4,
                    )
                    for v in range(nw_c):
                        w = c * CHW + v
                        for half, bd in ((0, bde), (1, bdo)):
                            nc.tensor.matmul(
                                out=ps[:],
                                lhsT=bd[:, w * 128 : w * 128 + 128],
                                rhs=g[
                                    :,
                                    v * PEW + half * EW : v * PEW + (half + 1) * EW,
                                ],
                                start=(w == 0 and half == 0),
                                stop=(w == nwin_t - 1 and half == 1),
                            )
                pooled = wk.tile([128, EW], F32, tag="pooled")
                nc.scalar.copy(out=pooled[:], in_=ps[:])

                # transpose 2-batch blocks, project, scale by 1/denom, store
                for k in range(4):
                    pst = psT.tile([128, 128], F32, tag="psTt")
                    nc.tensor.transpose(
                        out=pst[:],
                        in_=pooled[:, k * 128 : (k + 1) * 128],
                        identity=identity[:],
                    )
                    poolTb = wk.tile([128, 128], BF16, tag="poolTb")
                    nc.vector.tensor_copy(out=poolTb[:], in_=pst[:])
                    for h in range(2):
                        b = 2 * k + h
                        pso = psO.tile([128, COUT], F32, tag="psO")
                        nc.tensor.matmul(
                            out=pso[:],
                            lhsT=poolTb[64 * h : 64 * h + 64, :],
                            rhs=wresTb[64 * h : 64 * h + 64, :],
                            start=True,
                            stop=True,
                        )
                        outP = wk.tile([128, COUT], BF16, tag="outP")
                        nc.vector.tensor_scalar_mul(
                            outP[:], pso[:], recip[:, t : t + 1]
                        )
                        r0 = b * p.cpts_pad + t * 128
                        nc.sync.dma_start(out=out[r0 : r0 + 128, :], in_=outP[:])
    nc.compile()
    return nc


def host_prep(p: Params, in_pc_pad, ids, mask, pn, wres):
    """Per-core input maps.  Host work is layout marshalling only."""
    ids = np.asarray(ids).astype(np.int64)
    pn = np.asarray(pn, dtype=np.float32)
    mask = np.asarray(mask, dtype=np.float32)
    wres = np.asarray(wres, dtype=np.float32)
    x = np.asarray(in_pc_pad, dtype=np.float32)          # (B, 40001, 64)

    # pair table: xi[k] = [row 2k all batches | row 2k+1 all batches], bf16
    xp = np.concatenate([x, np.zeros((B, 1, CIN), np.float32)], axis=1)
    xi = np.ascontiguousarray(
        xp.transpose(1, 0, 2).reshape(2 * NPAIRS, EW).reshape(NPAIRS, PEW)
    ).astype(ml_dtypes.bfloat16)
    ident = np.eye(128, dtype=np.float32)

    in_maps = []
    for c in range(p.n_cores):
        lo = c * p.cpts

        def pad_pts(a, dtype):
            o = np.zeros((p.cpts_pad, MAXN), dtype=dtype)
            o[: p.cpts] = a[lo : lo + p.cpts]
            return o

        ids_c = pad_pts(ids, np.int64)
        ids_c[p.cpts :] = 2 * (NPAIRS - 1)               # pad points: valid pair
        pn_c = pad_pts(pn, np.float32)
        mask_c = pad_pts(mask, np.float32)
        par_c = (ids_c & 1).astype(np.float32)
        idx16 = (ids_c >> 1).astype(np.int16)

        # gather stream: tile t, window w, slot s=32q+m -> ids_c[t*128+4w+q, m]>>1
        flat = (
            idx16.reshape(p.ntl, NWIN, 4, MAXN)
            .transpose(0, 1, 2, 3)                       # (t, w, q, m)
            .reshape(p.ntl * NWIN * 128)
        )
        # wrapped-16 layout per call: idx i at [i % 16, i // 16]
        idx_w = np.zeros((128, p.ntl * NWIN * 8), np.int16)
        for t in range(p.ntl):
            for c in range(NCALL):
                w0 = c * CHW
                nwn = min(CHW, NWIN - w0)
                blk_flat = flat[t * 4096 + w0 * 128 : t * 4096 + (w0 + nwn) * 128]
                blk = blk_flat.reshape(nwn * 8, 16).T
                col0 = t * NWIN * 8 + c * CHW * 8
                idx_w[:, col0 : col0 + nwn * 8] = np.tile(blk, (8, 1))

        pnT = np.ascontiguousarray(np.tile(pn_c.T, (4, 1)))      # (128, cpts_pad)
        maskT = np.ascontiguousarray(np.tile(mask_c.T, (4, 1)))
        parT = np.ascontiguousarray(np.tile(par_c.T, (4, 1)))
        in_maps.append(
            {
                "xi": xi,
                "idxw": idx_w,
                "pnT": pnT,
                "maskT": maskT,
                "parT": parT,
                "pnN": pn_c,
                "maskN": mask_c,
                "wres": wres,
                "ident": ident,
            }
        )
    return in_maps


def assemble(p: Params, results):
    out = np.empty((B, p.pts, COUT), np.float32)
    for c in range(p.n_cores):
        got = np.asarray(results[c]["out"], dtype=np.float32).reshape(
            B, p.cpts_pad, COUT
        )
        out[:, c * p.cpts : (c + 1) * p.cpts, :] = got[:, : p.cpts, :]
    return out


_NC_CACHE = {}


def get_nc(p: Params):
    key = (p.pts, p.n_cores)
    if key not in _NC_CACHE:
        _NC_CACHE[key] = build_nc(p)
    return _NC_CACHE[key]


def kernel(in_pc_pad, neighbor_id_lstlst, neighbor_mask_lst, p_neighbors, weight_res):
    in_pc_pad = np.asarray(in_pc_pad)
    p = Params(pts=PTS, n_cores=in_pc_pad.shape[0])
    in_maps = host_prep(
        p, in_pc_pad, neighbor_id_lstlst, neighbor_mask_lst, p_neighbors, weight_res
    )
    nc = get_nc(p)
    res = run_bass_kernel_spmd(nc, in_maps, core_ids=list(range(p.n_cores)))
    return assemble(p, res.results)


# revision 21
# speedup vs baseline: 1.5065x; 1.5065x over previous
"""Trainium2 Bass kernel for nn_Pooling_Layer (GNN message-passing pooling).

Math (per batch element b):
    x = in_pc_pad[b] @ weight_res.T               # (N+1, 64) -> (N+1, 128) projection
    w = |p_neighbors| * mask; w /= w.sum(-1)+1e-8 # (P, 32) pooling weights
    out[b, p] = sum_m w[p, m] * x[id[p, m]]       # gather + weighted pool

We reorder: pool first in C_IN=64 space (gather is half the bytes), then
project pooled (P, 64) @ weight_res.T.  Normalization (divide by the weight
sum) is folded into the PSUM->SBUF copy after the projection.

Sharding: points are sharded across the 8 cores (1250 points each); every
core handles ALL batches for its points.  The gather table holds row PAIRS,
batch-interleaved, in bf16: xi[k] = [row 2k: b0..b7 x 64ch | row 2k+1:
b0..b7 x 64ch] (2KB rows).  Pairs keep the SWDGE gather indices int16-safe
(idx = id >> 1 <= 20000); one descriptor serves all 8 batches at a
DMA-efficient 2KB.  bf16 halves HBM gather traffic vs f32; the tolerance
(2e-2) dwarfs bf16 rounding (~0.5%).

Pooling runs on the TensorEngine: per 128-point tile, 64 accumulating bf16
matmuls (32 windows x even/odd half) into one (128 pts, 8b*64ch) PSUM bank.
lhsT is a block-diagonal weight matrix with a FIXED sparsity structure:
window w (slots = partitions: slot 32q+m = neighbor m of point 4w+q) puts
weight at [32q+m, 4w+q].  Even-half weights are |pn|*mask*(1-parity), odd
|pn|*mask*parity, so the wrong half of each gathered pair contributes 0.
The nonzero positions are identical for every tile, so the bd buffers are
zeroed once and only the values are rewritten per tile (tiny strided
copies).

Then per tile: 4 PE transposes (128pts, 2 batches*64ch) -> (128ch, 128pts),
8 projection matmuls lhsT=pooled^T (64,128) rhs=weight_res^T (64,128), and
the per-point 1/denom scale on the PSUM->SBUF copy.  Output is bf16,
upcast and re-assembled on the host.
"""

import numpy as np
import ml_dtypes

import concourse.bass as bass
import concourse.mybir as mybir
import concourse.tile as tile
from concourse import bacc, library_config
from concourse.bass_utils import run_bass_kernel_spmd

F32 = mybir.dt.float32
BF16 = mybir.dt.bfloat16
I16 = mybir.dt.int16

MAXN = 32
CIN = 64
COUT = 128
B = 8
IN_ROWS = 40001          # in_pc_pad rows (incl. pad row)
NPAIRS = 20001           # row pairs (rows padded to 40002)
EW = B * CIN             # interleaved single-row width (elements) = 512
PEW = 2 * EW             # pair-row width = 1024 elements (2KB bf16)
PTS = 10000
NWIN = 32                # windows (4-point groups) per 128-point tile
CHW = 8                  # windows per gather call (1024 idx)
NCALL = (NWIN + CHW - 1) // CHW   # gather call slots per tile


class Params:
    def __init__(self, pts=PTS, n_cores=8):
        self.pts = pts
        self.n_cores = n_cores
        self.cpts = pts // n_cores            # points per core (1250)
        self.ntl = (self.cpts + 127) // 128   # 128-point tiles per core (10)
        self.cpts_pad = self.ntl * 128        # 1280


def build_nc(p: Params):
    nc = bacc.Bacc(
        "TRN2",
        target_bir_lowering=False,
        debug=False,
        num_devices=p.n_cores,
        num_swdge_queues=4,
    )
    NTL = p.ntl
    xi = nc.dram_tensor("xi", [NPAIRS, PEW], BF16, kind="ExternalInput")
    idxw = nc.dram_tensor("idxw", [128, NTL * NWIN * 8], I16, kind="ExternalInput")
    pnT = nc.dram_tensor("pnT", [128, p.cpts_pad], F32, kind="ExternalInput")
    maskT = nc.dram_tensor("maskT", [128, p.cpts_pad], F32, kind="ExternalInput")
    parT = nc.dram_tensor("parT", [128, p.cpts_pad], F32, kind="ExternalInput")
    pnN = nc.dram_tensor("pnN", [p.cpts_pad, MAXN], F32, kind="ExternalInput")
    maskN = nc.dram_tensor("maskN", [p.cpts_pad, MAXN], F32, kind="ExternalInput")
    wres = nc.dram_tensor("wres", [COUT, CIN], F32, kind="ExternalInput")
    ident = nc.dram_tensor("ident", [128, 128], F32, kind="ExternalInput")
    out = nc.dram_tensor("out", [B * p.cpts_pad, COUT], BF16, kind="ExternalOutput")

    with tile.TileContext(nc) as tc:
        with (
            tc.tile_pool(name="const", bufs=1) as constp,
            tc.tile_pool(name="prep", bufs=1) as prep,
            tc.tile_pool(name="gather", bufs=6) as gp,
            tc.tile_pool(name="work", bufs=2) as wk,
            tc.tile_pool(name="psP", bufs=2, space="PSUM") as psP,
            tc.tile_pool(name="psT", bufs=2, space="PSUM") as psT,
            tc.tile_pool(name="psO", bufs=2, space="PSUM") as psO,
        ):
            nc.gpsimd.load_library(library_config.mlp)

            # ---- block-diag weight buffers: fixed sparsity, zeroed once ----
            # (first thing on DVE so tile 0's weight scatter isn't delayed)
            BDW = NWIN * 132  # 4224: bd[s, 132w + q] == lhsT col 4w+q of window w
            bd_bufs = []      # [t%2][half] ping-pong pairs
            for i in range(2):
                pair = []
                for half in range(2):
                    bdt = constp.tile([128, BDW], BF16, tag=f"bd{i}h{half}")
                    nc.vector.memset(bdt[:], 0.0)
                    pair.append(bdt)
                bd_bufs.append(pair)

            # ---- constants ----
            identity = constp.tile([128, 128], F32)
            nc.sync.dma_start(out=identity[:], in_=ident[:])
            wres_sb = constp.tile([COUT, CIN], F32)
            nc.sync.dma_start(out=wres_sb[:], in_=wres[:])
            psw = psT.tile([CIN, COUT], F32, tag="psTt")
            nc.tensor.transpose(out=psw[:], in_=wres_sb[:], identity=identity[:])
            # [i, o] = wres[o, i], replicated into both 64-partition halves so
            # the projection matmul's rhs base partition matches lhsT's
            wresTb = constp.tile([128, COUT], BF16)
            nc.vector.tensor_copy(out=wresTb[0:CIN, :], in_=psw[:])
            nc.vector.tensor_copy(out=wresTb[CIN : 2 * CIN, :], in_=psw[:])

            # idx loaded per tile so the first gather starts immediately
            idx_sb = constp.tile([128, NTL * NWIN * 8], I16)
            for t in range(NTL):
                c0 = t * NWIN * 8
                nc.sync.dma_start(
                    out=idx_sb[:, c0 : c0 + NWIN * 8],
                    in_=idxw[:, c0 : c0 + NWIN * 8],
                )

            # ---- per-point reciprocal denominators: recip[p, t] ----
            prodN = prep.tile([128, NTL * MAXN], F32)
            nc.sync.dma_start(
                out=prodN[:].rearrange("p (t m) -> p t m", m=MAXN),
                in_=pnN[:].rearrange("(t p) m -> p t m", p=128),
            )
            maskN_sb = prep.tile([128, NTL * MAXN], F32)
            nc.sync.dma_start(
                out=maskN_sb[:].rearrange("p (t m) -> p t m", m=MAXN),
                in_=maskN[:].rearrange("(t p) m -> p t m", p=128),
            )
            nc.vector.tensor_tensor(
                out=prodN[:], in0=prodN[:], in1=maskN_sb[:], op=mybir.AluOpType.mult
            )
            denom = constp.tile([128, NTL], F32)
            nc.vector.tensor_reduce(
                out=denom[:],
                in_=prodN[:].rearrange("p (t m) -> p t m", m=MAXN),
                op=mybir.AluOpType.add,
                axis=mybir.AxisListType.X,
                apply_absolute_value=True,
            )
            nc.vector.tensor_scalar_add(denom[:], denom[:], 1e-8)
            recip = constp.tile([128, NTL], F32)
            nc.vector.reciprocal(out=recip[:], in_=denom[:])

            # ---- pooling weights in (32q+m, pt) layout ----
            # wsel0 = |pn|*mask*(1-par)   (even half)
            # wsel1 = |pn|*mask*par       (odd half)
            pnT_sb = prep.tile([128, p.cpts_pad], F32)
            maskT_sb = prep.tile([128, p.cpts_pad], F32)
            parT_sb = prep.tile([128, p.cpts_pad], F32)
            nc.sync.dma_start(out=pnT_sb[:], in_=pnT[:])
            nc.sync.dma_start(out=maskT_sb[:], in_=maskT[:])
            nc.sync.dma_start(out=parT_sb[:], in_=parT[:])
            wsel0 = prep.tile([128, p.cpts_pad], F32)
            wsel1 = prep.tile([128, p.cpts_pad], F32)
            nc.scalar.activation(
                out=wsel0[:], in_=pnT_sb[:], func=mybir.ActivationFunctionType.Abs
            )
            nc.vector.tensor_tensor(
                out=wsel0[:], in0=wsel0[:], in1=maskT_sb[:], op=mybir.AluOpType.mult
            )
            nc.vector.tensor_tensor(
                out=wsel1[:], in0=wsel0[:], in1=parT_sb[:], op=mybir.AluOpType.mult
            )
            nc.vector.tensor_tensor(
                out=wsel0[:], in0=wsel0[:], in1=wsel1[:], op=mybir.AluOpType.subtract
            )

            # ---- main loop over 128-point tiles ----
            for t in range(NTL):
                # windows with at least one real (non-pad) point
                real_pts = min(128, p.cpts - t * 128)
                nwin_t = (real_pts + 3) // 4
                # scatter this tile's weights onto the fixed block-diag slots
                bde, bdo = bd_bufs[t % 2]
                for bd, src in ((bde, wsel0), (bdo, wsel1)):
                    bdv = bd[:].rearrange("p (w c) -> p w c", c=132)
                    sv = src[:, t * 128 : (t + 1) * 128].rearrange(
                        "p (w four) -> p w four", four=4
                    )
                    for q in range(4):
                        nc.vector.tensor_copy(
                            out=bdv[32 * q : 32 * q + 32, :, q],
                            in_=sv[32 * q : 32 * q + 32, :, q],
                        )

                # gather + pool in chunks of 8 windows (1024 idx per call)
                ps = psP.tile([128, EW], F32, tag="ps")
                for c in range((nwin_t + CHW - 1) // CHW):
                    nw_c = min(CHW, nwin_t - c * CHW)
                    g = gp.tile([128, CHW * PEW], BF16, tag="g")
                    call = t * NCALL + c
                    col0 = t * NWIN * 8 + c * CHW * 8
                    nc.gpsimd.dma_gather(
                        g[:, : nw_c * PEW].rearrange("p (v e) -> p v e", e=PEW),
                        xi[:],
                        idx_sb[:, col0 : col0 + nw_c * 8],
                        nw_c * 128,
                        nw_c * 128,
                        PEW,
                        queue_num=call % 4,
                    )
                    for v in range(nw_c):
                        w = c * CHW + v
                        for half, bd in ((0, bde), (1, bdo)):
                            nc.tensor.matmul(
                                out=ps[:],
                                lhsT=bd[:, w * 128 : w * 128 + 128],
                                rhs=g[
                                    :,
                                    v * PEW + half * EW : v * PEW + (half + 1) * EW,
                                ],
                                start=(w == 0 and half == 0),
                                stop=(w == nwin_t - 1 and half == 1),
                            )
                pooled = wk.tile([128, EW], F32, tag="pooled")
                nc.scalar.copy(out=pooled[:], in_=ps[:])

                # 4 transposes back-to-back (identity stays stationary), then
                # Act-engine casts, 8 projections, Act scale-copies, stores
                psts = []
                for k in range(4):
                    pst = psT.tile([128, 128], F32, tag="psTt")
                    nc.tensor.transpose(
                        out=pst[:],
                        in_=pooled[:, k * 128 : (k + 1) * 128],
                        identity=identity[:],
                    )
                    psts.append(pst)
                poolTb = wk.tile([128, 512], BF16, tag="poolTb")
                for k in range(4):
                    nc.scalar.copy(
                        out=poolTb[:, k * 128 : (k + 1) * 128], in_=psts[k][:]
                    )
                for b in range(8):
                    k, h = b // 2, b % 2
                    pso = psO.tile([128, COUT], F32, tag="psO")
                    nc.tensor.matmul(
                        out=pso[:],
                        lhsT=poolTb[64 * h : 64 * h + 64, k * 128 : (k + 1) * 128],
                        rhs=wresTb[64 * h : 64 * h + 64, :],
                        start=True,
                        stop=True,
                    )
                    outP = wk.tile([128, COUT], BF16, tag=f"outP{b % 2}")
                    nc.scalar.activation(
                        out=outP[:],
                        in_=pso[:],
                        func=mybir.ActivationFunctionType.Copy,
                        scale=recip[:, t : t + 1],
                    )
                    r0 = b * p.cpts_pad + t * 128
                    nc.sync.dma_start(out=out[r0 : r0 + 128, :], in_=outP[:])
    nc.compile()
    return nc


def host_prep(p: Params, in_pc_pad, ids, mask, pn, wres):
    """Per-core input maps.  Host work is layout marshalling only."""
    ids = np.asarray(ids).astype(np.int64)
    pn = np.asarray(pn, dtype=np.float32)
    mask = np.asarray(mask, dtype=np.float32)
    wres = np.asarray(wres, dtype=np.float32)
    x = np.asarray(in_pc_pad, dtype=np.float32)          # (B, 40001, 64)

    # pair table: xi[k] = [row 2k all batches | row 2k+1 all batches], bf16
    xp = np.concatenate([x, np.zeros((B, 1, CIN), np.float32)], axis=1)
    xi = np.ascontiguousarray(
        xp.transpose(1, 0, 2).reshape(2 * NPAIRS, EW).reshape(NPAIRS, PEW)
    ).astype(ml_dtypes.bfloat16)
    ident = np.eye(128, dtype=np.float32)

    in_maps = []
    for c in range(p.n_cores):
        lo = c * p.cpts

        def pad_pts(a, dtype):
            o = np.zeros((p.cpts_pad, MAXN), dtype=dtype)
            o[: p.cpts] = a[lo : lo + p.cpts]
            return o

        ids_c = pad_pts(ids, np.int64)
        ids_c[p.cpts :] = 2 * (NPAIRS - 1)               # pad points: valid pair
        pn_c = pad_pts(pn, np.float32)
        mask_c = pad_pts(mask, np.float32)
        par_c = (ids_c & 1).astype(np.float32)
        idx16 = (ids_c >> 1).astype(np.int16)

        # gather stream: tile t, window w, slot s=32q+m -> ids_c[t*128+4w+q, m]>>1
        flat = (
            idx16.reshape(p.ntl, NWIN, 4, MAXN)
            .transpose(0, 1, 2, 3)                       # (t, w, q, m)
            .reshape(p.ntl * NWIN * 128)
        )
        # wrapped-16 layout per call: idx i at [i % 16, i // 16]
        idx_w = np.zeros((128, p.ntl * NWIN * 8), np.int16)
        for t in range(p.ntl):
            for c in range(NCALL):
                w0 = c * CHW
                nwn = min(CHW, NWIN - w0)
                blk_flat = flat[t * 4096 + w0 * 128 : t * 4096 + (w0 + nwn) * 128]
                blk = blk_flat.reshape(nwn * 8, 16).T
                col0 = t * NWIN * 8 + c * CHW * 8
                idx_w[:, col0 : col0 + nwn * 8] = np.tile(blk, (8, 1))

        pnT = np.ascontiguousarray(np.tile(pn_c.T, (4, 1)))      # (128, cpts_pad)
        maskT = np.ascontiguousarray(np.tile(mask_c.T, (4, 1)))
        parT = np.ascontiguousarray(np.tile(par_c.T, (4, 1)))
        in_maps.append(
            {
                "xi": xi,
                "idxw": idx_w,
                "pnT": pnT,
                "maskT": maskT,
                "parT": parT,
                "pnN": pn_c,
                "maskN": mask_c,
                "wres": wres,
                "ident": ident,
            }
        )
    return in_maps


def assemble(p: Params, results):
    out = np.empty((B, p.pts, COUT), np.float32)
    for c in range(p.n_cores):
        got = np.asarray(results[c]["out"], dtype=np.float32).reshape(
            B, p.cpts_pad, COUT
        )
        out[:, c * p.cpts : (c + 1) * p.cpts, :] = got[:, : p.cpts, :]
    return out


_NC_CACHE = {}


def get_nc(p: Params):
    key = (p.pts, p.n_cores)
    if key not in _NC_CACHE:
        _NC_CACHE[key] = build_nc(p)
    return _NC_CACHE[key]


def kernel(in_pc_pad, neighbor_id_lstlst, neighbor_mask_lst, p_neighbors, weight_res):
    in_pc_pad = np.asarray(in_pc_pad)
    p = Params(pts=PTS, n_cores=in_pc_pad.shape[0])
    in_maps = host_prep(
        p, in_pc_pad, neighbor_id_lstlst, neighbor_mask_lst, p_neighbors, weight_res
    )
    nc = get_nc(p)
    res = run_bass_kernel_spmd(nc, in_maps, core_ids=list(range(p.n_cores)))
    return assemble(p, res.results)


# revision 22
# speedup vs baseline: 1.7959x; 1.1921x over previous
"""Trainium2 Bass kernel for nn_Pooling_Layer (GNN message-passing pooling):
parity-split zero-waste gather + TensorEngine pooling + fused projection.

Math (per batch b): x = in_pc_pad[b] @ weight_res.T; w = |pn|*mask
normalized; out[b,p] = sum_m w[p,m] * x[id[p,m]].  We pool first in
C_IN=64 space, then project; normalization is folded into the final
PSUM->SBUF scale-copy.  Points are sharded across 8 cores; each core
handles all batches for its 1250 points.  Tables are batch-interleaved
bf16 rows (64ch x 8b = 1KB), split into separate contiguous even-id and
odd-id tables so SWDGE int16 indices (id >> 1 <= 20000) reach every row
with zero gather waste.

Each 128-point tile's 4096 slots are partitioned by neighbor-id parity
and gathered from the matching table (1KB contiguous descriptors, calls
striped across the 4 SWDGE queues in lane order).  Streams are padded to
a uniform window count across cores so the program stays SPMD.

The slot->point mapping becomes data-dependent, so the block-diagonal
lhsT is replaced by per-window weight matrices W_w[s, p] =
|pn|*mask * (p == point_of_slot), built on-device with one fused DVE
tensor_scalar (op0=is_equal against a host iota plane, op1=mult by the
weight value).  Streams are padded (weight 0, idx = pad pair) to a
uniform per-call/window count across all cores so the program stays SPMD.
"""

import numpy as np
import ml_dtypes

import concourse.bass as bass
import concourse.mybir as mybir
import concourse.tile as tile
from concourse import bacc, library_config
from concourse.bass_utils import run_bass_kernel_spmd

F32 = mybir.dt.float32
BF16 = mybir.dt.bfloat16
I16 = mybir.dt.int16

MAXN = 32
CIN = 64
COUT = 128
B = 8
NPAIRS = 20001
EW = B * CIN             # 512 elements (1KB bf16) per gathered row
PEW = 2 * EW             # pair-row width in the table
PTS = 10000


class Params:
    def __init__(self, pts=PTS, n_cores=8, new=16, now=16):
        self.pts = pts
        self.n_cores = n_cores
        self.cpts = pts // n_cores
        self.ntl = (self.cpts + 127) // 128
        self.cpts_pad = self.ntl * 128
        self.new = new            # even windows per tile (uniform, padded)
        self.now = now            # odd windows per tile
        self.nw = new + now       # total windows per tile


def _calls(nwin):
    """Split nwin 128-slot windows into gather calls of <= 8 windows."""
    out = []
    w = 0
    while w < nwin:
        n = min(8, nwin - w)
        out.append((w, n))
        w += n
    return out


def build_nc(p: Params):
    nc = bacc.Bacc(
        "TRN2",
        target_bir_lowering=False,
        debug=False,
        num_devices=p.n_cores,
        num_swdge_queues=4,
    )
    NTL, NW = p.ntl, p.nw
    ecalls, ocalls = _calls(p.new), _calls(p.now)
    ncall_t = len(ecalls) + len(ocalls)
    idx_cols = NW * 8          # idx words per tile (NW*128/16)

    xiE = nc.dram_tensor("xiE", [NPAIRS, EW], BF16, kind="ExternalInput")
    xiO = nc.dram_tensor("xiO", [NPAIRS, EW], BF16, kind="ExternalInput")
    idxw = nc.dram_tensor("idxw", [128, NTL * idx_cols], I16, kind="ExternalInput")
    colP = nc.dram_tensor("colP", [128, NTL * NW], F32, kind="ExternalInput")
    pnP = nc.dram_tensor("pnP", [128, NTL * NW], F32, kind="ExternalInput")
    maskP = nc.dram_tensor("maskP", [128, NTL * NW], F32, kind="ExternalInput")
    pnN = nc.dram_tensor("pnN", [p.cpts_pad, MAXN], F32, kind="ExternalInput")
    maskN = nc.dram_tensor("maskN", [p.cpts_pad, MAXN], F32, kind="ExternalInput")
    wres = nc.dram_tensor("wres", [COUT, CIN], F32, kind="ExternalInput")
    ident = nc.dram_tensor("ident", [128, 128], F32, kind="ExternalInput")
    iota = nc.dram_tensor("iota", [128, 128], F32, kind="ExternalInput")
    out = nc.dram_tensor("out", [B * p.cpts_pad, COUT], BF16, kind="ExternalOutput")

    with tile.TileContext(nc) as tc:
        with (
            tc.tile_pool(name="const", bufs=1) as constp,
            tc.tile_pool(name="prep", bufs=1) as prep,
            tc.tile_pool(name="gather", bufs=2) as gp,
            tc.tile_pool(name="wmat", bufs=2) as wm,
            tc.tile_pool(name="work", bufs=2) as wk,
            tc.tile_pool(name="psP", bufs=2, space="PSUM") as psP,
            tc.tile_pool(name="psT", bufs=2, space="PSUM") as psT,
            tc.tile_pool(name="psO", bufs=2, space="PSUM") as psO,
        ):
            nc.gpsimd.load_library(library_config.mlp)

            # ---- constants ----
            identity = constp.tile([128, 128], F32)
            nc.sync.dma_start(out=identity[:], in_=ident[:])
            iotaP = constp.tile([128, 128], F32)
            nc.sync.dma_start(out=iotaP[:], in_=iota[:])
            wres_sb = constp.tile([COUT, CIN], F32)
            nc.sync.dma_start(out=wres_sb[:], in_=wres[:])
            psw = psT.tile([CIN, COUT], F32, tag="psTt")
            nc.tensor.transpose(out=psw[:], in_=wres_sb[:], identity=identity[:])
            wresTb = constp.tile([128, COUT], BF16)
            nc.vector.tensor_copy(out=wresTb[0:CIN, :], in_=psw[:])
            nc.vector.tensor_copy(out=wresTb[CIN : 2 * CIN, :], in_=psw[:])

            idx_sb = constp.tile([128, NTL * idx_cols], I16)
            nc.sync.dma_start(out=idx_sb[:], in_=idxw[:])

            # ---- per-point reciprocal denominators: recip[p, t] ----
            prodN = prep.tile([128, NTL * MAXN], F32)
            nc.sync.dma_start(
                out=prodN[:].rearrange("p (t m) -> p t m", m=MAXN),
                in_=pnN[:].rearrange("(t p) m -> p t m", p=128),
            )
            maskN_sb = prep.tile([128, NTL * MAXN], F32)
            nc.sync.dma_start(
                out=maskN_sb[:].rearrange("p (t m) -> p t m", m=MAXN),
                in_=maskN[:].rearrange("(t p) m -> p t m", p=128),
            )
            nc.vector.tensor_tensor(
                out=prodN[:], in0=prodN[:], in1=maskN_sb[:], op=mybir.AluOpType.mult
            )
            denom = constp.tile([128, NTL], F32)
            nc.vector.tensor_reduce(
                out=denom[:],
                in_=prodN[:].rearrange("p (t m) -> p t m", m=MAXN),
                op=mybir.AluOpType.add,
                axis=mybir.AxisListType.X,
                apply_absolute_value=True,
            )
            nc.vector.tensor_scalar_add(denom[:], denom[:], 1e-8)
            recip = constp.tile([128, NTL], F32)
            nc.vector.reciprocal(out=recip[:], in_=denom[:])

            # ---- per-slot weight values |pnP|*maskP (permuted layout) ----
            pnP_sb = prep.tile([128, NTL * NW], F32)
            maskP_sb = prep.tile([128, NTL * NW], F32)
            colP_sb = prep.tile([128, NTL * NW], F32)
            nc.sync.dma_start(out=pnP_sb[:], in_=pnP[:])
            nc.sync.dma_start(out=maskP_sb[:], in_=maskP[:])
            nc.sync.dma_start(out=colP_sb[:], in_=colP[:])
            wvP = prep.tile([128, NTL * NW], F32)
            nc.scalar.activation(
                out=wvP[:], in_=pnP_sb[:], func=mybir.ActivationFunctionType.Abs
            )
            nc.vector.tensor_tensor(
                out=wvP[:], in0=wvP[:], in1=maskP_sb[:], op=mybir.AluOpType.mult
            )


            # ---- main loop over 128-point tiles ----
            gcall = 0
            for t in range(NTL):
                # gather: evens then odds, into one (128, NW*512) tile
                g = gp.tile([128, NW * EW], BF16, tag="g")
                cidx = 0
                for view, calls, wbase in ((xiE[:], ecalls, 0), (xiO[:], ocalls, p.new)):
                    for (w0, nwn) in calls:
                        nidx = nwn * 128
                        col0 = t * idx_cols + cidx
                        nc.gpsimd.dma_gather(
                            g[
                                :, (wbase + w0) * EW : (wbase + w0 + nwn) * EW
                            ].rearrange("p (v e) -> p v e", e=EW),
                            view,
                            idx_sb[:, col0 : col0 + nidx // 16],
                            nidx,
                            nidx,
                            EW,
                            queue_num=gcall % 4,
                        )
                        cidx += nidx // 16
                        gcall += 1

                # build ALL W matrices for the tile with two broadcast
                # tensor_tensor ops: W[s, w, p] = (iota[s,p]==col[s,w])*wv[s,w]
                wmat = wm.tile([128, NW * 128], BF16, tag="wmat")
                wmv = wmat[:].rearrange("p (w c) -> p w c", c=128)
                iview = (
                    iotaP[:]
                    .rearrange("p (o c) -> p o c", o=1)
                    .to_broadcast([128, NW, 128])
                )
                cview = (
                    colP_sb[:, t * NW : (t + 1) * NW]
                    .rearrange("p (w o) -> p w o", o=1)
                    .to_broadcast([128, NW, 128])
                )
                wview = (
                    wvP[:, t * NW : (t + 1) * NW]
                    .rearrange("p (w o) -> p w o", o=1)
                    .to_broadcast([128, NW, 128])
                )
                nc.vector.tensor_tensor(
                    out=wmv, in0=iview, in1=cview, op=mybir.AluOpType.is_equal
                )
                nc.vector.tensor_tensor(
                    out=wmv, in0=wmv, in1=wview, op=mybir.AluOpType.mult
                )
                ps = psP.tile([128, EW], F32, tag="ps")
                for w in range(NW):
                    nc.tensor.matmul(
                        out=ps[:],
                        lhsT=wmat[:, w * 128 : (w + 1) * 128],
                        rhs=g[:, w * EW : (w + 1) * EW],
                        start=(w == 0),
                        stop=(w == NW - 1),
                    )
                pooled = wk.tile([128, EW], F32, tag="pooled")
                nc.scalar.copy(out=pooled[:], in_=ps[:])

                # 4 transposes back-to-back, Act casts, projections, stores
                psts = []
                for k in range(4):
                    pst = psT.tile([128, 128], F32, tag="psTt")
                    nc.tensor.transpose(
                        out=pst[:],
                        in_=pooled[:, k * 128 : (k + 1) * 128],
                        identity=identity[:],
                    )
                    psts.append(pst)
                poolTb = wk.tile([128, 512], BF16, tag="poolTb")
                for k in range(4):
                    nc.scalar.copy(
                        out=poolTb[:, k * 128 : (k + 1) * 128], in_=psts[k][:]
                    )
                for b in range(8):
                    k, h = b // 2, b % 2
                    pso = psO.tile([128, COUT], F32, tag="psO")
                    nc.tensor.matmul(
                        out=pso[:],
                        lhsT=poolTb[64 * h : 64 * h + 64, k * 128 : (k + 1) * 128],
                        rhs=wresTb[64 * h : 64 * h + 64, :],
                        start=True,
                        stop=True,
                    )
                    outP = wk.tile([128, COUT], BF16, tag=f"outP{b % 2}")
                    nc.scalar.activation(
                        out=outP[:],
                        in_=pso[:],
                        func=mybir.ActivationFunctionType.Copy,
                        scale=recip[:, t : t + 1],
                    )
                    r0 = b * p.cpts_pad + t * 128
                    nc.sync.dma_start(out=out[r0 : r0 + 128, :], in_=outP[:])
    nc.compile()
    return nc


def host_prep(p, in_pc_pad, ids, mask, pn, wres):
    """Returns (params, in_maps): window counts are data-dependent."""
    ids = np.asarray(ids).astype(np.int64)
    pn = np.asarray(pn, dtype=np.float32)
    mask = np.asarray(mask, dtype=np.float32)
    wres = np.asarray(wres, dtype=np.float32)
    x = np.asarray(in_pc_pad, dtype=np.float32)

    xp = np.concatenate([x, np.zeros((B, 1, CIN), np.float32)], axis=1)
    xflat = xp.transpose(1, 0, 2).reshape(2 * NPAIRS, EW)
    xiE = np.ascontiguousarray(xflat[0::2]).astype(ml_dtypes.bfloat16)
    xiO = np.ascontiguousarray(xflat[1::2]).astype(ml_dtypes.bfloat16)
    ident = np.eye(128, dtype=np.float32)
    iota = np.tile(np.arange(128, dtype=np.float32), (128, 1))

    # ---- per (core, tile): build parity-split slot streams ----
    n_cores = p.n_cores
    cores = []
    new_max = now_max = 0
    for c in range(n_cores):
        lo = c * p.cpts

        def pad_pts(a, dtype):
            o = np.zeros((p.cpts_pad, MAXN), dtype=dtype)
            o[: p.cpts] = a[lo : lo + p.cpts]
            return o

        ids_c = pad_pts(ids, np.int64)
        ids_c[p.cpts :] = 2 * (NPAIRS - 1)
        pn_c = pad_pts(pn, np.float32)
        mask_c = pad_pts(mask, np.float32)
        mask_c[p.cpts :] = 0          # pad points contribute zero weight
        plocal = np.repeat(np.arange(128), MAXN).reshape(128, MAXN)
        tiles = []
        for t in range(p.ntl):
            pts = slice(t * 128, (t + 1) * 128)
            idt = ids_c[pts]                     # (128, 32)
            par = (idt & 1).astype(bool)
            real = ((np.arange(128) + t * 128) < p.cpts)[:, None]

            # boolean indexing flattens row-major = point-major, m-minor
            def stream(sel):
                return (
                    (idt[sel] >> 1).astype(np.int16),
                    plocal[sel].astype(np.float32),
                    pn_c[pts][sel],
                    mask_c[pts][sel],
                )

            eidx, ecol, epn, emask = stream(~par & real)
            oidx, ocol, opn, omask = stream(par & real)
            tiles.append((eidx, ecol, epn, emask, oidx, ocol, opn, omask))
            new_max = max(new_max, (len(eidx) + 127) // 128)
            now_max = max(now_max, (len(oidx) + 127) // 128)
        cores.append((tiles, pn_c, mask_c))

    p2 = Params(pts=p.pts, n_cores=n_cores, new=new_max, now=now_max)
    NW = p2.nw
    idx_cols = NW * 8
    ecalls, ocalls = _calls(p2.new), _calls(p2.now)

    in_maps = []
    for c in range(n_cores):
        tiles, pn_c, mask_c = cores[c]
        idx_w = np.zeros((128, p2.ntl * idx_cols), np.int16)
        colP = np.zeros((128, p2.ntl * NW), np.float32)
        pnP = np.zeros((128, p2.ntl * NW), np.float32)
        maskP = np.zeros((128, p2.ntl * NW), np.float32)
        for t in range(p2.ntl):
            eidx, ecol, epn, emask, oidx, ocol, opn, omask = tiles[t]
            for (sidx, scol, spn, smask, nwn, wbase, calls) in (
                (eidx, ecol, epn, emask, p2.new, 0, ecalls),
                (oidx, ocol, opn, omask, p2.now, p2.new, ocalls),
            ):
                L = nwn * 128
                fi = np.full(L, NPAIRS - 1, np.int16)
                fc = np.zeros(L, np.float32)
                fp = np.zeros(L, np.float32)
                fm = np.zeros(L, np.float32)
                fi[: len(sidx)] = sidx
                fc[: len(sidx)] = scol
                fp[: len(sidx)] = spn
                fm[: len(sidx)] = smask
                # per-window planes
                wslice = slice(t * NW + wbase, t * NW + wbase + nwn)
                colP[:, wslice] = fc.reshape(nwn, 128).T
                pnP[:, wslice] = fp.reshape(nwn, 128).T
                maskP[:, wslice] = fm.reshape(nwn, 128).T
                # wrapped idx per call
                cidx = wbase * 8
                for (w0, ncw) in calls:
                    nidx = ncw * 128
                    blk = fi[w0 * 128 : w0 * 128 + nidx].reshape(nidx // 16, 16).T
                    col0 = t * idx_cols + cidx
                    idx_w[:, col0 : col0 + nidx // 16] = np.tile(blk, (8, 1))
                    cidx += nidx // 16
        in_maps.append(
            {
                "xiE": xiE,
                "xiO": xiO,
                "idxw": idx_w,
                "colP": colP,
                "pnP": pnP,
                "maskP": maskP,
                "pnN": pn_c,
                "maskN": mask_c,
                "wres": wres,
                "ident": ident,
                "iota": iota,
            }
        )
    return p2, in_maps


def assemble(p: Params, results):
    out = np.empty((B, p.pts, COUT), np.float32)
    for c in range(p.n_cores):
        got = np.asarray(results[c]["out"], dtype=np.float32).reshape(
            B, p.cpts_pad, COUT
        )
        out[:, c * p.cpts : (c + 1) * p.cpts, :] = got[:, : p.cpts, :]
    return out


_NC_CACHE = {}


def get_nc(p: Params):
    key = (p.pts, p.n_cores, p.new, p.now)
    if key not in _NC_CACHE:
        _NC_CACHE[key] = build_nc(p)
    return _NC_CACHE[key]


def kernel(in_pc_pad, neighbor_id_lstlst, neighbor_mask_lst, p_neighbors, weight_res):
    in_pc_pad = np.asarray(in_pc_pad)
    p0 = Params(pts=PTS, n_cores=in_pc_pad.shape[0])
    p, in_maps = host_prep(
        p0, in_pc_pad, neighbor_id_lstlst, neighbor_mask_lst, p_neighbors, weight_res
    )
    nc = get_nc(p)
    res = run_bass_kernel_spmd(nc, in_maps, core_ids=list(range(p.n_cores)))
    return assemble(p, res.results)


# revision 24
# speedup vs baseline: 1.8276x; 1.0177x over previous
"""Trainium2 Bass kernel for nn_Pooling_Layer (GNN message-passing pooling):
parity-split zero-waste gather + TensorEngine pooling + fused projection.

Math (per batch b): x = in_pc_pad[b] @ weight_res.T; w = |pn|*mask
normalized; out[b,p] = sum_m w[p,m] * x[id[p,m]].  We pool first in
C_IN=64 space, then project; normalization is folded into the final
PSUM->SBUF scale-copy.  Points are sharded across 8 cores; each core
handles all batches for its 1250 points.  Tables are batch-interleaved
bf16 rows (64ch x 8b = 1KB), split into separate contiguous even-id and
odd-id tables so SWDGE int16 indices (id >> 1 <= 20000) reach every row
with zero gather waste.

Each 128-point tile's 4096 slots are partitioned by neighbor-id parity
and gathered from the matching table (1KB contiguous descriptors, calls
striped across the 4 SWDGE queues in lane order).  Streams are padded to
a uniform window count across cores so the program stays SPMD.

The slot->point mapping becomes data-dependent, so the block-diagonal
lhsT is replaced by per-window weight matrices W_w[s, p] =
|pn|*mask * (p == point_of_slot), built on-device with one fused DVE
tensor_scalar (op0=is_equal against a host iota plane, op1=mult by the
weight value).  Streams are padded (weight 0, idx = pad pair) to a
uniform per-call/window count across all cores so the program stays SPMD.
"""

import numpy as np
import ml_dtypes

import concourse.bass as bass
import concourse.mybir as mybir
import concourse.tile as tile
from concourse import bacc, library_config
from concourse.bass_utils import run_bass_kernel_spmd

F32 = mybir.dt.float32
BF16 = mybir.dt.bfloat16
I16 = mybir.dt.int16

MAXN = 32
CIN = 64
COUT = 128
B = 8
NPAIRS = 20001
OVROWS = 2048            # per-core overflow rows appended to each table
NROWS_T = NPAIRS + OVROWS
EW = B * CIN             # 512 elements (1KB bf16) per gathered row
PEW = 2 * EW             # pair-row width in the table
PTS = 10000


class Params:
    def __init__(self, pts=PTS, n_cores=8, new=16, now=16):
        self.pts = pts
        self.n_cores = n_cores
        self.cpts = pts // n_cores
        self.ntl = (self.cpts + 127) // 128
        self.cpts_pad = self.ntl * 128
        self.new = new            # even windows per tile (uniform, padded)
        self.now = now            # odd windows per tile
        self.nw = new + now       # total windows per tile


def _calls(nwin):
    """Split nwin 128-slot windows into gather calls of <= 8 windows."""
    out = []
    w = 0
    while w < nwin:
        n = min(8, nwin - w)
        out.append((w, n))
        w += n
    return out


def build_nc(p: Params):
    nc = bacc.Bacc(
        "TRN2",
        target_bir_lowering=False,
        debug=False,
        num_devices=p.n_cores,
        num_swdge_queues=4,
    )
    NTL, NW = p.ntl, p.nw
    ecalls, ocalls = _calls(p.new), _calls(p.now)
    ncall_t = len(ecalls) + len(ocalls)
    idx_cols = NW * 8          # idx words per tile (NW*128/16)

    xiE = nc.dram_tensor("xiE", [NROWS_T, EW], BF16, kind="ExternalInput")
    xiO = nc.dram_tensor("xiO", [NROWS_T, EW], BF16, kind="ExternalInput")
    idxw = nc.dram_tensor("idxw", [128, NTL * idx_cols], I16, kind="ExternalInput")
    colP = nc.dram_tensor("colP", [128, NTL * NW], F32, kind="ExternalInput")
    pnP = nc.dram_tensor("pnP", [128, NTL * NW], F32, kind="ExternalInput")
    maskP = nc.dram_tensor("maskP", [128, NTL * NW], F32, kind="ExternalInput")
    pnN = nc.dram_tensor("pnN", [p.cpts_pad, MAXN], F32, kind="ExternalInput")
    maskN = nc.dram_tensor("maskN", [p.cpts_pad, MAXN], F32, kind="ExternalInput")
    wres = nc.dram_tensor("wres", [COUT, CIN], F32, kind="ExternalInput")
    ident = nc.dram_tensor("ident", [128, 128], F32, kind="ExternalInput")
    iota = nc.dram_tensor("iota", [128, 128], F32, kind="ExternalInput")
    out = nc.dram_tensor("out", [B * p.cpts_pad, COUT], BF16, kind="ExternalOutput")

    with tile.TileContext(nc) as tc:
        with (
            tc.tile_pool(name="const", bufs=1) as constp,
            tc.tile_pool(name="prep", bufs=1) as prep,
            tc.tile_pool(name="gather", bufs=2) as gp,
            tc.tile_pool(name="wmat", bufs=2) as wm,
            tc.tile_pool(name="work", bufs=2) as wk,
            tc.tile_pool(name="psP", bufs=2, space="PSUM") as psP,
            tc.tile_pool(name="psT", bufs=2, space="PSUM") as psT,
            tc.tile_pool(name="psO", bufs=2, space="PSUM") as psO,
        ):
            nc.gpsimd.load_library(library_config.mlp)

            # ---- constants ----
            identity = constp.tile([128, 128], F32)
            nc.sync.dma_start(out=identity[:], in_=ident[:])
            iotaP = constp.tile([128, 128], F32)
            nc.sync.dma_start(out=iotaP[:], in_=iota[:])
            wres_sb = constp.tile([COUT, CIN], F32)
            nc.sync.dma_start(out=wres_sb[:], in_=wres[:])
            psw = psT.tile([CIN, COUT], F32, tag="psTt")
            nc.tensor.transpose(out=psw[:], in_=wres_sb[:], identity=identity[:])
            wresTb = constp.tile([128, COUT], BF16)
            nc.vector.tensor_copy(out=wresTb[0:CIN, :], in_=psw[:])
            nc.vector.tensor_copy(out=wresTb[CIN : 2 * CIN, :], in_=psw[:])

            idx_sb = constp.tile([128, NTL * idx_cols], I16)
            nc.sync.dma_start(out=idx_sb[:], in_=idxw[:])

            # ---- per-point reciprocal denominators: recip[p, t] ----
            prodN = prep.tile([128, NTL * MAXN], F32)
            nc.sync.dma_start(
                out=prodN[:].rearrange("p (t m) -> p t m", m=MAXN),
                in_=pnN[:].rearrange("(t p) m -> p t m", p=128),
            )
            maskN_sb = prep.tile([128, NTL * MAXN], F32)
            nc.sync.dma_start(
                out=maskN_sb[:].rearrange("p (t m) -> p t m", m=MAXN),
                in_=maskN[:].rearrange("(t p) m -> p t m", p=128),
            )
            nc.vector.tensor_tensor(
                out=prodN[:], in0=prodN[:], in1=maskN_sb[:], op=mybir.AluOpType.mult
            )
            denom = constp.tile([128, NTL], F32)
            nc.vector.tensor_reduce(
                out=denom[:],
                in_=prodN[:].rearrange("p (t m) -> p t m", m=MAXN),
                op=mybir.AluOpType.add,
                axis=mybir.AxisListType.X,
                apply_absolute_value=True,
            )
            nc.vector.tensor_scalar_add(denom[:], denom[:], 1e-8)
            recip = constp.tile([128, NTL], F32)
            nc.vector.reciprocal(out=recip[:], in_=denom[:])

            # ---- per-slot weight values |pnP|*maskP (permuted layout) ----
            pnP_sb = prep.tile([128, NTL * NW], F32)
            maskP_sb = prep.tile([128, NTL * NW], F32)
            colP_sb = prep.tile([128, NTL * NW], F32)
            nc.sync.dma_start(out=pnP_sb[:], in_=pnP[:])
            nc.sync.dma_start(out=maskP_sb[:], in_=maskP[:])
            nc.sync.dma_start(out=colP_sb[:], in_=colP[:])
            wvP = prep.tile([128, NTL * NW], F32)
            nc.scalar.activation(
                out=wvP[:], in_=pnP_sb[:], func=mybir.ActivationFunctionType.Abs
            )
            nc.vector.tensor_tensor(
                out=wvP[:], in0=wvP[:], in1=maskP_sb[:], op=mybir.AluOpType.mult
            )


            # ---- main loop over 128-point tiles ----
            gcall = 0
            for t in range(NTL):
                # gather: evens then odds, into one (128, NW*512) tile
                g = gp.tile([128, NW * EW], BF16, tag="g")
                cidx = 0
                for view, calls, wbase in ((xiE[:], ecalls, 0), (xiO[:], ocalls, p.new)):
                    for (w0, nwn) in calls:
                        nidx = nwn * 128
                        col0 = t * idx_cols + cidx
                        nc.gpsimd.dma_gather(
                            g[
                                :, (wbase + w0) * EW : (wbase + w0 + nwn) * EW
                            ].rearrange("p (v e) -> p v e", e=EW),
                            view,
                            idx_sb[:, col0 : col0 + nidx // 16],
                            nidx,
                            nidx,
                            EW,
                            queue_num=gcall % 4,
                        )
                        cidx += nidx // 16
                        gcall += 1

                # build ALL W matrices for the tile with two broadcast
                # tensor_tensor ops: W[s, w, p] = (iota[s,p]==col[s,w])*wv[s,w]
                wmat = wm.tile([128, NW * 128], BF16, tag="wmat")
                wmv = wmat[:].rearrange("p (w c) -> p w c", c=128)
                iview = (
                    iotaP[:]
                    .rearrange("p (o c) -> p o c", o=1)
                    .to_broadcast([128, NW, 128])
                )
                cview = (
                    colP_sb[:, t * NW : (t + 1) * NW]
                    .rearrange("p (w o) -> p w o", o=1)
                    .to_broadcast([128, NW, 128])
                )
                wview = (
                    wvP[:, t * NW : (t + 1) * NW]
                    .rearrange("p (w o) -> p w o", o=1)
                    .to_broadcast([128, NW, 128])
                )
                nc.vector.tensor_tensor(
                    out=wmv, in0=iview, in1=cview, op=mybir.AluOpType.is_equal
                )
                nc.vector.tensor_tensor(
                    out=wmv, in0=wmv, in1=wview, op=mybir.AluOpType.mult
                )
                ps = psP.tile([128, EW], F32, tag="ps")
                for w in range(NW):
                    nc.tensor.matmul(
                        out=ps[:],
                        lhsT=wmat[:, w * 128 : (w + 1) * 128],
                        rhs=g[:, w * EW : (w + 1) * EW],
                        start=(w == 0),
                        stop=(w == NW - 1),
                    )
                pooled = wk.tile([128, EW], F32, tag="pooled")
                nc.scalar.copy(out=pooled[:], in_=ps[:])

                # 4 transposes back-to-back, Act casts, projections, stores
                psts = []
                for k in range(4):
                    pst = psT.tile([128, 128], F32, tag="psTt")
                    nc.tensor.transpose(
                        out=pst[:],
                        in_=pooled[:, k * 128 : (k + 1) * 128],
                        identity=identity[:],
                    )
                    psts.append(pst)
                poolTb = wk.tile([128, 512], BF16, tag="poolTb")
                for k in range(4):
                    nc.scalar.copy(
                        out=poolTb[:, k * 128 : (k + 1) * 128], in_=psts[k][:]
                    )
                for b in range(8):
                    k, h = b // 2, b % 2
                    pso = psO.tile([128, COUT], F32, tag="psO")
                    nc.tensor.matmul(
                        out=pso[:],
                        lhsT=poolTb[64 * h : 64 * h + 64, k * 128 : (k + 1) * 128],
                        rhs=wresTb[64 * h : 64 * h + 64, :],
                        start=True,
                        stop=True,
                    )
                    outP = wk.tile([128, COUT], BF16, tag=f"outP{b % 2}")
                    nc.scalar.activation(
                        out=outP[:],
                        in_=pso[:],
                        func=mybir.ActivationFunctionType.Copy,
                        scale=recip[:, t : t + 1],
                    )
                    r0 = b * p.cpts_pad + t * 128
                    nc.sync.dma_start(out=out[r0 : r0 + 128, :], in_=outP[:])
    nc.compile()
    return nc


def host_prep(p, in_pc_pad, ids, mask, pn, wres):
    """Returns (params, in_maps): window counts are data-dependent."""
    ids = np.asarray(ids).astype(np.int64)
    pn = np.asarray(pn, dtype=np.float32)
    mask = np.asarray(mask, dtype=np.float32)
    wres = np.asarray(wres, dtype=np.float32)
    x = np.asarray(in_pc_pad, dtype=np.float32)

    xp = np.concatenate([x, np.zeros((B, 1, CIN), np.float32)], axis=1)
    xflat = xp.transpose(1, 0, 2).reshape(2 * NPAIRS, EW)
    xiE = np.ascontiguousarray(xflat[0::2]).astype(ml_dtypes.bfloat16)
    xiO = np.ascontiguousarray(xflat[1::2]).astype(ml_dtypes.bfloat16)
    ident = np.eye(128, dtype=np.float32)
    iota = np.tile(np.arange(128, dtype=np.float32), (128, 1))

    # ---- per (core, tile): build parity-split slot streams ----
    n_cores = p.n_cores
    cores = []
    new_max = now_max = 0
    for c in range(n_cores):
        lo = c * p.cpts

        def pad_pts(a, dtype):
            o = np.zeros((p.cpts_pad, MAXN), dtype=dtype)
            o[: p.cpts] = a[lo : lo + p.cpts]
            return o

        ids_c = pad_pts(ids, np.int64)
        ids_c[p.cpts :] = 2 * (NPAIRS - 1)
        pn_c = pad_pts(pn, np.float32)
        mask_c = pad_pts(mask, np.float32)
        mask_c[p.cpts :] = 0          # pad points contribute zero weight
        plocal = np.repeat(np.arange(128), MAXN).reshape(128, MAXN)
        tiles = []
        # per-core overflow regions appended to the OTHER parity's table:
        # relocated even rows go into xiO2's tail, odd rows into xiE2's tail
        ovE = []   # odd rows appended to xiE (gathered via the E stream)
        ovO = []   # even rows appended to xiO (gathered via the O stream)
        for t in range(p.ntl):
            pts = slice(t * 128, (t + 1) * 128)
            idt = ids_c[pts]                     # (128, 32)
            par = (idt & 1).astype(bool)
            real = ((np.arange(128) + t * 128) < p.cpts)[:, None]

            # boolean indexing flattens row-major = point-major, m-minor
            def stream(sel):
                return [
                    (idt[sel] >> 1).astype(np.int64),
                    plocal[sel].astype(np.float32),
                    pn_c[pts][sel],
                    mask_c[pts][sel],
                ]

            e = stream(~par & real)
            o = stream(par & real)
            # rebalance: move the majority parity's overflow (beyond 2048
            # slots) into the other stream; its rows are appended to the
            # other table so the other-table idx can reach them
            if len(e[0]) > 2048:
                src_s, dst_s, ov, base = e, o, ovO, NPAIRS + len(ovO)
            else:
                src_s, dst_s, ov, base = o, e, ovE, NPAIRS + len(ovE)
            nmove = max(0, len(src_s[0]) - 2048)
            assert len(dst_s[0]) + nmove <= 2048
            if nmove:
                moved_rows = src_s[0][2048:]     # table-row ids (id>>1)
                ov.extend(moved_rows.tolist())
                for j in range(4):
                    tail = src_s[j][2048:]
                    if j == 0:
                        tail = base + np.arange(nmove, dtype=np.int64)
                    dst_s[j] = np.concatenate([dst_s[j], tail])
                    src_s[j] = src_s[j][:2048]
            tiles.append((*[a.astype(np.int16) if i == 0 else a
                            for s in (e, o) for i, a in enumerate(s)],))
        assert len(ovE) <= OVROWS and len(ovO) <= OVROWS, (len(ovE), len(ovO))
        cores.append((tiles, pn_c, mask_c, np.array(ovE, np.int64),
                      np.array(ovO, np.int64)))
        new_max = now_max = 16

    p2 = Params(pts=p.pts, n_cores=n_cores, new=new_max, now=now_max)
    NW = p2.nw
    idx_cols = NW * 8
    ecalls, ocalls = _calls(p2.new), _calls(p2.now)

    in_maps = []
    for c in range(n_cores):
        tiles, pn_c, mask_c, ovE, ovO = cores[c]
        xiE2 = np.zeros((NROWS_T, EW), ml_dtypes.bfloat16)
        xiE2[:NPAIRS] = xiE
        if len(ovE):
            xiE2[NPAIRS : NPAIRS + len(ovE)] = xiO[ovE]   # odd rows
        xiO2 = np.zeros((NROWS_T, EW), ml_dtypes.bfloat16)
        xiO2[:NPAIRS] = xiO
        if len(ovO):
            xiO2[NPAIRS : NPAIRS + len(ovO)] = xiE[ovO]   # even rows
        idx_w = np.zeros((128, p2.ntl * idx_cols), np.int16)
        colP = np.zeros((128, p2.ntl * NW), np.float32)
        pnP = np.zeros((128, p2.ntl * NW), np.float32)
        maskP = np.zeros((128, p2.ntl * NW), np.float32)
        for t in range(p2.ntl):
            eidx, ecol, epn, emask, oidx, ocol, opn, omask = tiles[t]
            for (sidx, scol, spn, smask, nwn, wbase, calls) in (
                (eidx, ecol, epn, emask, p2.new, 0, ecalls),
                (oidx, ocol, opn, omask, p2.now, p2.new, ocalls),
            ):
                L = nwn * 128
                fi = np.full(L, NPAIRS - 1, np.int16)
                fc = np.zeros(L, np.float32)
                fp = np.zeros(L, np.float32)
                fm = np.zeros(L, np.float32)
                fi[: len(sidx)] = sidx
                fc[: len(sidx)] = scol
                fp[: len(sidx)] = spn
                fm[: len(sidx)] = smask
                # per-window planes
                wslice = slice(t * NW + wbase, t * NW + wbase + nwn)
                colP[:, wslice] = fc.reshape(nwn, 128).T
                pnP[:, wslice] = fp.reshape(nwn, 128).T
                maskP[:, wslice] = fm.reshape(nwn, 128).T
                # wrapped idx per call
                cidx = wbase * 8
                for (w0, ncw) in calls:
                    nidx = ncw * 128
                    blk = fi[w0 * 128 : w0 * 128 + nidx].reshape(nidx // 16, 16).T
                    col0 = t * idx_cols + cidx
                    idx_w[:, col0 : col0 + nidx // 16] = np.tile(blk, (8, 1))
                    cidx += nidx // 16
        in_maps.append(
            {
                "xiE": xiE2,
                "xiO": xiO2,
                "idxw": idx_w,
                "colP": colP,
                "pnP": pnP,
                "maskP": maskP,
                "pnN": pn_c,
                "maskN": mask_c,
                "wres": wres,
                "ident": ident,
                "iota": iota,
            }
        )
    return p2, in_maps


def assemble(p: Params, results):
    out = np.empty((B, p.pts, COUT), np.float32)
    for c in range(p.n_cores):
        got = np.asarray(results[c]["out"], dtype=np.float32).reshape(
            B, p.cpts_pad, COUT
        )
        out[:, c * p.cpts : (c + 1) * p.cpts, :] = got[:, : p.cpts, :]
    return out


_NC_CACHE = {}


def get_nc(p: Params):
    key = (p.pts, p.n_cores, p.new, p.now)
    if key not in _NC_CACHE:
        _NC_CACHE[key] = build_nc(p)
    return _NC_CACHE[key]


def kernel(in_pc_pad, neighbor_id_lstlst, neighbor_mask_lst, p_neighbors, weight_res):
    in_pc_pad = np.asarray(in_pc_pad)
    p0 = Params(pts=PTS, n_cores=in_pc_pad.shape[0])
    p, in_maps = host_prep(
        p0, in_pc_pad, neighbor_id_lstlst, neighbor_mask_lst, p_neighbors, weight_res
    )
    nc = get_nc(p)
    res = run_bass_kernel_spmd(nc, in_maps, core_ids=list(range(p.n_cores)))
    return assemble(p, res.results)
